# revision 1
# baseline (speedup 1.0000x reference)
"""Bass/Tile kernel for nn_Attention2d: 2D attention block with channel-LN,
qkv 1x1 conv, depthwise 3x3 convs, relative-position-bias attention, out proj.

Sharding: data-parallel over batch, 2 batches per core, 8 cores, no collectives.

Algorithm (validated in np_check.py; fp16 operand sim => ~3.5e-4 rel err):
  - LN scale folded into w_qkv columns (host); q-scale D^-0.5 folded into
    dw_w_q / dw_b_q (host).
  - LN: mean/var via ones-column matmuls over channel chunks; per-spatial
    broadcast of r and mu*r via gpsimd.partition_broadcast.
  - depthwise conv: 9 diagonal matmuls (diag built via affine_select) over
    zero-padded 34x34 spatial maps, accumulated in PSUM.
  - attention transposed: S^T[j,i] matmuls from channel-major conv outputs
    (head pairs row-packed in the PE array); P^T = exp(S^T) * expBias^T
    (bias exp-ed + gathered host-side, fp16); O^T via V_aug (ones column)
    so column sums Z arrive in row 64; normalize by 1/Z via reciprocal +
    partition_broadcast; out proj K=128 over packed head pairs.

Schedule: two batches are software-pipelined by interleaving the emission
order (Tile's static scheduler follows trace priority):
  phase1: LN+qkv+conv(b0)
  phase2: attention(b0) interleaved with LN+qkv+conv(b1)
  phase3: attention(b1) interleaved with outproj(b0)
  phase4: outproj(b1)
"""

import numpy as np

import concourse.bass as bass
import concourse.mybir as mybir
import concourse.tile as tile
from concourse import bacc
from concourse.masks import make_identity

F32 = mybir.dt.float32
F16 = mybir.dt.float16

B, C, S = 16, 512, 32
H, D = 8, 64
INNER = H * D  # 512
SEQ = S * S  # 1024
SP = S + 2  # padded spatial edge
EPS = 1e-5
SCALE = D**-0.5
N_CORES = 8
BB = B // N_CORES  # batches per core
NCHUNK = C // 128  # 4 channel chunks
NJT = SEQ // 128  # 8 seq j-tiles
TAPS = [(dx, dy) for dx in (-1, 0, 1) for dy in (-1, 0, 1)]
ABLATE = set()  # ablation paths disabled in final kernel


class Ctx:
    pass


def build_program(num_devices=N_CORES, rep=1):
    nc = bacc.Bacc("TRN2", target_bir_lowering=False, debug=False,
                   num_devices=num_devices)
    g = Ctx()
    g.nc = nc

    g.x_d = nc.dram_tensor("x16", [BB, NCHUNK, 128, SEQ], F16, kind="ExternalInput")
    g.wq_d = nc.dram_tensor("wqkvT", [NCHUNK, 128, 3 * INNER], F16,
                            kind="ExternalInput")
    g.wo_d = nc.dram_tensor("woutT", [NCHUNK, 128, C], F16, kind="ExternalInput")
    g.dwv_d = nc.dram_tensor("dwv", [128, 3, 9, NCHUNK], F16, kind="ExternalInput")
    g.dwb_d = nc.dram_tensor("dwb", [128, 3, NCHUNK], F32, kind="ExternalInput")
    g.eb_d = nc.dram_tensor("ebT", [H, NJT, 2, 128, 512], F16, kind="ExternalInput")
    g.y_d = nc.dram_tensor("y", [BB, NCHUNK, 128, SEQ], F32, kind="ExternalOutput")

    with tile.TileContext(nc) as tc:
        g.tc = tc
        with (
            tc.tile_pool(name="singles", bufs=1) as singles,
            tc.tile_pool(name="ebpool", bufs=5) as ebpool,
            tc.tile_pool(name="sc", bufs=1) as sc,
            tc.tile_pool(name="bat", bufs=1) as bat,
            tc.tile_pool(name="psum", bufs=1, space="PSUM") as psum,
        ):
            g.ebpool, g.sc, g.bat, g.psum = ebpool, sc, bat, psum
            g.wq_sb = singles.tile([128, NCHUNK, 3 * INNER], F16, tag="wq")
            nc.sync.dma_start(out=g.wq_sb,
                              in_=g.wq_d.ap().rearrange("k p o -> p k o"))
            g.wo_sb = singles.tile([128, NCHUNK, C], F16, tag="wo")
            nc.sync.dma_start(out=g.wo_sb,
                              in_=g.wo_d.ap().rearrange("k p o -> p k o"))
            g.dwv_sb = singles.tile([128, 3, 9, NCHUNK], F16, tag="dwv")
            nc.sync.dma_start(out=g.dwv_sb, in_=g.dwv_d.ap())
            g.dwb_sb = singles.tile([128, 3, NCHUNK], F32, tag="dwb")
            nc.sync.dma_start(out=g.dwb_sb, in_=g.dwb_d.ap())
            g.ident = singles.tile([128, 128], F16, tag="ident")
            make_identity(nc, g.ident[:, :])
            g.ones_col = singles.tile([128, 1], F16, tag="ones")
            nc.vector.memset(g.ones_col, 1.0)
            g.eps_sb = singles.tile([128, 1], F32, tag="eps")
            nc.vector.memset(g.eps_sb, EPS)

            from contextlib import ExitStack
            rep_ctx = ExitStack()
            if rep > 1:
                rep_ctx.enter_context(tc.For_i(0, rep, 1))
            st = [Ctx(), Ctx()]  # per-batch tile refs

            def chain(*gens):
                for gg in gens:
                    yield from gg

            # LN + qkv for both batches (sequential emission; scheduler
            # still overlaps b1's scalar chain with b0's qkv matmuls).
            # conv(b, chunk0) is interleaved into b's qkv emission once its
            # three o-blocks are out.
            dgs = {}
            for b01 in (0, 1):
                st[b01].ocmh = bat.tile([128, NCHUNK, SEQ], F16, tag="ocmh",
                                        bufs=2, name=f"ocmh{b01}")
            for b01 in (0, 1):
                conv0 = _conv_gen(g, b01, 0, st[b01], dgs)

                def pull0(conv0=conv0):
                    try:
                        next(conv0)
                    except StopIteration:
                        pass

                for _ in _compute_gen(g, b01, st[b01], pull0):
                    pass
                for _ in conv0:
                    pass
            # attention chunk k interleaves the convs for chunk k+1 at
            # per-jt grain; a drain barrier before attn(k+1) guarantees its
            # inputs are fully emitted first (Tile deps follow trace order).
            dgs = {} if dgs is None else dgs
            conv_chains = {kk: chain(_conv_gen(g, 0, kk, st[0], dgs),
                                     _conv_gen(g, 1, kk, st[1], dgs))
                           for kk in range(1, NCHUNK)}
            for k in range(NCHUNK):
                partner = conv_chains.get(k + 1)

                def pull(partner=partner):
                    if partner is not None:
                        try:
                            next(partner)
                        except StopIteration:
                            pass

                units = 0
                for _ in _attn_gen(g, k, st, pull):
                    units += 1
                    if k == NCHUNK - 1 and units == 1:
                        # n=0 of last chunk done: emit outproj for n=0
                        if partner is not None:
                            for _ in partner:
                                pass
                        for _ in chain(_outproj_gen(g, 0, st[0], 0),
                                       _outproj_gen(g, 1, st[1], 0)):
                            pass
                if partner is not None:
                    for _ in partner:
                        pass
            for _ in chain(_outproj_gen(g, 0, st[0], 1), _outproj_gen(g, 1, st[1], 1)):
                pass
            rep_ctx.close()

    nc.compile()
    return nc


def _interleave(main_gen, partner_gen, ratio):
    """Pull `ratio` partner units per main unit; drain both."""
    done = False
    for _ in main_gen:
        for _ in range(ratio):
            if done:
                break
            try:
                next(partner_gen)
            except StopIteration:
                done = True
    for _ in partner_gen:
        pass


def _compute_gen(g, b, s, pull=None):
    """LN + qkv + conv for batch b. Yields between units."""
    nc, tc, bat, sc = g.nc, g.tc, g.bat, g.sc

    xc = bat.tile([128, NCHUNK, SEQ], F16, tag="xc", name="xc")
    nc.sync.dma_start(out=xc, in_=g.x_d.ap()[b].rearrange("k p s -> p k s"))

    rb = bat.tile([128, SEQ], F16, tag="rb", name="rb")
    murb = bat.tile([128, SEQ], F16, tag="murb", name="murb")

    # ---- LN stats + scalar chain per n-half ----
    if True:
        statp = g.psum
        for n in range(2):
            nh = slice(512 * n, 512 * (n + 1))
            stx = statp.tile([1, 512], F32, tag="mm", bufs=2, name="stx")
            for k in range(NCHUNK):
                nc.tensor.matmul(stx[:, :], g.ones_col[:, :], xc[:, k, nh],
                                 start=(k == 0), stop=(k == NCHUNK - 1))
            stxx = statp.tile([1, 512], F32, tag="mm", bufs=2, name="stxx")
            for k in range(NCHUNK):
                xsqk = bat.tile([128, 512], F16, tag="xsq", bufs=2, name="xsqk")
                nc.vector.tensor_mul(xsqk, xc[:, k, nh], xc[:, k, nh])
                nc.tensor.matmul(stxx[:, :], g.ones_col[:, :], xsqk[:, :],
                                 start=(k == 0), stop=(k == NCHUNK - 1))
            mu = sc.tile([1, 512], F32, tag="mu", name="mu")
            nc.scalar.mul(out=mu, in_=stx[:, :], mul=1.0 / C)
            ex2 = sc.tile([1, 512], F32, tag="ex2", name="ex2")
            nc.scalar.mul(out=ex2, in_=stxx[:, :], mul=1.0 / C)
            musq = sc.tile([1, 512], F32, tag="musq", name="musq")
            nc.vector.tensor_mul(musq, mu, mu)
            var = sc.tile([1, 512], F32, tag="var", name="var")
            nc.vector.tensor_sub(var, ex2, musq)
            sd = sc.tile([1, 512], F32, tag="sd", name="sd")
            nc.scalar.activation(out=sd, in_=var,
                                 func=mybir.ActivationFunctionType.Sqrt,
                                 bias=g.eps_sb[0:1, :], scale=1.0)
            r_row = sc.tile([1, 512], F32, tag="r", name="r_row")
            nc.vector.reciprocal(out=r_row, in_=sd)
            mur_row = sc.tile([1, 512], F32, tag="mur", name="mur_row")
            nc.vector.tensor_mul(mur_row, mu, r_row)
            r16 = sc.tile([1, 512], F16, tag="r16", name="r16")
            nc.scalar.copy(out=r16, in_=r_row)
            mur16 = sc.tile([1, 512], F16, tag="mur16", name="mur16")
            nc.scalar.copy(out=mur16, in_=mur_row)
            nc.gpsimd.partition_broadcast(rb[:, nh], r16[:, :])
            nc.gpsimd.partition_broadcast(murb[:, nh], mur16[:, :])
            yield

    # ---- xn = x*rb - murb (in-place second step) ----
    xn = bat.tile([128, NCHUNK, SEQ], F16, tag="xn", name="xn")
    for k in range(NCHUNK):
        nc.vector.tensor_mul(xn[:, k, :], xc[:, k, :], rb)
        nc.vector.tensor_sub(xn[:, k, :], xn[:, k, :], murb)
    yield

    # ---- conv output buffers (allocated early; conv(chunk0) interleaves
    # into the qkv emission below via pull) ----
    qc = bat.tile([128, NCHUNK, SEQ], F16, tag="qc", bufs=2, name="qc")
    kc = bat.tile([128, NCHUNK, SEQ], F16, tag="kc", bufs=2, name="kc")
    vaug = bat.tile([128, NJT, H, 66], F16, tag="vaug", bufs=2, name="vaug")
    nc.vector.memset(vaug, 1.0)  # col 64 = ones; cols 0..63 overwritten
    s.qc, s.kc, s.vaug = qc, kc, vaug

    # ---- qkv matmul into zero-padded 34x34 maps ----
    qkv_sb = bat.tile([128, 12, SP * SP], F16, tag="qkv", bufs=2, name="qkv_sb")
    q3all = qkv_sb[:, :, :].rearrange("p o (x y) -> p o x y", x=SP)
    nc.vector.memset(q3all[:, :, 0, :], 0.0)
    nc.vector.memset(q3all[:, :, SP - 1, :], 0.0)
    nc.vector.memset(q3all[:, :, 1:SP - 1, 0], 0.0)
    nc.vector.memset(q3all[:, :, 1:SP - 1, SP - 1], 0.0)
    s.qkv_sb = qkv_sb
    if True:
        qp = g.psum
        for oi, o in enumerate([0, 4, 8, 1, 5, 9, 2, 6, 10, 3, 7, 11]):
            o3 = qkv_sb[:, o, :].rearrange("p (x y) -> p x y", x=SP)
            for n in range(2):
                if pull is not None and oi >= 3:
                    pull()
                ps = qp.tile([128, 512], F32, tag="mm", bufs=2, name="qkvps")
                for k in range(NCHUNK):
                    nc.tensor.matmul(
                        ps[:, :],
                        g.wq_sb[:, k, o * 128:(o + 1) * 128],
                        xn[:, k, n * 512:(n + 1) * 512],
                        start=(k == 0), stop=(k == NCHUNK - 1),
                    )
                nc.scalar.copy(
                    out=o3[:, 1 + 16 * n:17 + 16 * n, 1:33],
                    in_=ps[:, :].rearrange("p (x y) -> p x y", x=16))
            yield



def _conv_gen(g, b, k, s, dgs=None):
    """Depthwise conv for chunk k of batch b (heads 2k, 2k+1)."""
    nc = g.nc
    qc, kc, vaug, qkv_sb = s.qc, s.kc, s.vaug, s.qkv_sb
    if "noconv" in ABLATE:
        for t in range(3):
            src3 = qkv_sb[:, 4 * t + k, :].rearrange("p (x y) -> p x y", x=SP)
            interior = src3[:, 1:33, 1:33]
            if t == 0:
                nc.vector.tensor_copy(
                    qc[:, k, :].rearrange("p (x y) -> p x y", x=32), interior)
            elif t == 1:
                nc.vector.tensor_copy(
                    kc[:, k, :].rearrange("p (x y) -> p x y", x=32), interior)
            else:
                vcm = g.bat.tile([128, SEQ], F16, tag="vcm", bufs=2, name="vcm")
                nc.vector.tensor_copy(
                    vcm[:, :].rearrange("p (x y) -> p x y", x=32), interior)
                for jt in range(NJT):
                    tr = g.psum.tile([128, 128], F16, tag="mm", bufs=2, name="tr")
                    nc.tensor.transpose(
                        tr[:, :], vcm[:, jt * 128:(jt + 1) * 128], g.ident[:, :])
                    nc.vector.tensor_copy(vaug[:, jt, 2 * k, 0:64], tr[:, 0:64])
                    nc.vector.tensor_copy(vaug[:, jt, 2 * k + 1, 0:64],
                                          tr[:, 64:128])
            yield
        return
    for t in range(3):
        if dgs is not None and (k, t) in dgs:
            dg = dgs[k, t]
        else:
            dg = g.bat.tile([128, 9, 128], F16, tag="dg", bufs=4, name="dg")
            for tp in range(9):
                wv = g.dwv_sb[:, t, tp, k:k + 1]
                wv_b = bass.AP(tensor=wv.tensor, offset=wv.offset,
                               ap=[wv.ap[0], [0, 128]])
                nc.gpsimd.affine_select(
                    out=dg[:, tp, :], in_=wv_b,
                    compare_op=mybir.AluOpType.is_equal,
                    fill=0.0, base=0, pattern=[[-1, 128]],
                    channel_multiplier=1,
                )
            if dgs is not None:
                dgs[k, t] = dg
        src3 = qkv_sb[:, 4 * t + k, :].rearrange("p (x y) -> p x y", x=SP)
        bias_ap = g.dwb_sb[:, t, k:k + 1]
        if t == 2:
            vcm = g.bat.tile([128, SEQ], F16, tag="vcm", bufs=2, name="vcm")
        for n in range(2):
            cv = g.psum.tile([128, 512], F32, tag="mm", bufs=2, name="cv")
            for tp, (dx, dy) in enumerate(TAPS):
                nc.tensor.matmul(
                    cv[:, :],
                    dg[:, tp, :],
                    src3[:, 1 + dx + 16 * n:17 + dx + 16 * n, 1 + dy:33 + dy],
                    start=(tp == 0), stop=(tp == 8),
                    skip_group_check=True,
                )
            nh = slice(512 * n, 512 * (n + 1))
            if t == 0:
                nc.vector.tensor_scalar_add(qc[:, k, nh], cv[:, :], bias_ap)
            elif t == 1:
                nc.vector.tensor_scalar_add(kc[:, k, nh], cv[:, :], bias_ap)
            else:
                nc.vector.tensor_scalar_add(vcm[:, nh], cv[:, :], bias_ap)
            yield
        if t == 2:
            for jt in range(NJT):
                tr = g.psum.tile([128, 128], F16, tag="mm", bufs=2, name="tr")
                nc.tensor.transpose(
                    tr[:, :], vcm[:, jt * 128:(jt + 1) * 128], g.ident[:, :])
                nc.vector.tensor_copy(vaug[:, jt, 2 * k, 0:64], tr[:, 0:64])
                nc.vector.tensor_copy(vaug[:, jt, 2 * k + 1, 0:64],
                                      tr[:, 64:128])
                if jt % 3 == 2:
                    yield
            yield


def _attn_gen(g, k, st01, pull=None):
    """Attention for chunk k (heads 2k,2k+1), BOTH batches per unit so each
    bias tile is loaded once. Yields per n-half; calls pull() per jt to
    interleave partner work at fine grain."""
    nc = g.nc
    for n in range(2):
        nh = slice(512 * n, 512 * (n + 1))
        o_ps = {}
        for b01 in (0, 1):
            for h01 in (0, 1):
                o_ps[b01, h01] = g.psum.tile(
                    [65, 512], F32, tag=f"o{b01}{h01}", name=f"o_ps{b01}{h01}")
        for jt in range(NJT):
            for h01 in (0, 1):
                if pull is not None:
                    pull()
                h = 2 * k + h01
                pr = slice(64 * h01, 64 * h01 + 64)
                if "noeb" not in ABLATE:
                    eb_sb = g.ebpool.tile([128, 512], F16, tag="eb", name="ebt")
                    nc.sync.dma_start(out=eb_sb, in_=g.eb_d.ap()[h, jt, n])
                for b01 in (0, 1):
                    s = st01[b01]
                    st_ps = g.psum.tile([128, 512], F32, tag="st", bufs=2,
                                        name="st_ps")
                    nc.tensor.matmul(
                        st_ps[:, :],
                        s.kc[pr, k, jt * 128:(jt + 1) * 128],
                        s.qc[pr, k, nh],
                    )
                    if "noexp" in ABLATE:
                        pt = eb_sb
                    elif "noeb" in ABLATE:
                        pt = g.bat.tile([128, 512], F16, tag="pt", bufs=3,
                                        name="pt")
                        nc.scalar.activation(
                            out=pt, in_=st_ps[:, :],
                            func=mybir.ActivationFunctionType.Exp)
                    else:
                        p0 = g.bat.tile([128, 512], F16, tag="p0", bufs=3,
                                        name="p0")
                        nc.scalar.activation(
                            out=p0, in_=st_ps[:, :],
                            func=mybir.ActivationFunctionType.Exp)
                        pt = g.bat.tile([128, 512], F16, tag="pt", bufs=3,
                                        name="pt")
                        nc.vector.tensor_mul(pt, p0, eb_sb)
                    nc.tensor.matmul(
                        o_ps[b01, h01][:, :],
                        s.vaug[:, jt, h, 0:65],
                        pt[:, :],
                        start=(jt == 0), stop=(jt == NJT - 1),
                        skip_group_check=True,
                    )
        for b01 in (0, 1):
            for h01 in (0, 1):
                s = st01[b01]
                zrow = g.sc.tile([1, 512], F32, tag="zrow", name="zrow")
                nc.scalar.copy(out=zrow, in_=o_ps[b01, h01][64:65, :])
                zrec = g.sc.tile([1, 512], F32, tag="zrec", name="zrec")
                nc.vector.reciprocal(out=zrec, in_=zrow)
                rz = g.bat.tile([64, 512], F32, tag="rz", bufs=1, name="rz")
                nc.gpsimd.partition_broadcast(rz[:, :], zrec[:, :])
                nc.vector.tensor_mul(
                    s.ocmh[64 * h01:64 * h01 + 64, k, nh],
                    o_ps[b01, h01][0:64, :], rz)
        yield


def _outproj_gen(g, b, s, n_only=None):
    nc, tc = g.nc, g.tc
    ocmh = s.ocmh
    if True:
        outp = g.psum
        for o in range(NCHUNK):
            for n in range(2):
                if n_only is not None and n != n_only:
                    continue
                ps = outp.tile([128, 512], F32, tag="mm", bufs=2, name="ops")
                for hp in range(NCHUNK):
                    nc.tensor.matmul(
                        ps[:, :],
                        g.wo_sb[:, hp, o * 128:(o + 1) * 128],
                        ocmh[:, hp, n * 512:(n + 1) * 512],
                        start=(hp == 0), stop=(hp == NCHUNK - 1),
                    )
                ysb = g.bat.tile([128, 512], F32, tag="ysb", bufs=2, name="ysb")
                nc.scalar.copy(out=ysb, in_=ps[:, :])
                nc.sync.dma_start(
                    out=g.y_d.ap()[b, o, :, n * 512:(n + 1) * 512], in_=ysb)
                yield


# ---------------- host-side preparation ----------------

def prep_inputs(x, scale, w_qkv, dw_w_q, dw_b_q, dw_w_k, dw_b_k, dw_w_v, dw_b_v,
                w_out, pos_bias, pos_indices):
    """Full inputs -> list of per-core in_maps (numpy)."""
    x = np.asarray(x, np.float32)
    scale = np.asarray(scale, np.float32).reshape(C)
    w_qkv = np.asarray(w_qkv, np.float32) * scale[None, :]
    dw_w = np.stack([np.asarray(dw_w_q) * SCALE, np.asarray(dw_w_k),
                     np.asarray(dw_w_v)]).astype(np.float32)
    dw_b = np.stack([np.asarray(dw_b_q) * SCALE, np.asarray(dw_b_k),
                     np.asarray(dw_b_v)]).astype(np.float32)
    w_out = np.asarray(w_out, np.float32)

    wqkvT = np.ascontiguousarray(
        w_qkv.T.reshape(NCHUNK, 128, 3 * INNER)).astype(np.float16)
    woutT = np.ascontiguousarray(
        w_out.T.reshape(NCHUNK, 128, C)).astype(np.float16)
    dwv = dw_w.reshape(3, NCHUNK, 128, 9).transpose(2, 0, 3, 1)
    dwv = np.ascontiguousarray(dwv).astype(np.float16)
    dwb = np.ascontiguousarray(dw_b.reshape(3, NCHUNK, 128).transpose(2, 0, 1))
    dwb = dwb.astype(np.float32)
    eb = np.exp(np.asarray(pos_bias, np.float32))[np.asarray(pos_indices)]
    ebT = np.ascontiguousarray(eb.transpose(2, 1, 0)).astype(np.float16)
    ebT = np.ascontiguousarray(
        ebT.reshape(H, NJT, 128, 2, 512).transpose(0, 1, 3, 2, 4))

    x16 = x.reshape(N_CORES, BB, NCHUNK, 128, SEQ).astype(np.float16)

    shared = {"wqkvT": wqkvT, "woutT": woutT, "dwv": dwv, "dwb": dwb, "ebT": ebT}
    return [dict(shared, x16=x16[c]) for c in range(N_CORES)]


def gather_output(results):
    y = np.stack([r["y"] for r in results])
    return y.reshape(B, C, S, S)


# ---------------- harness entry point ----------------

_cache = {}


def kernel(**inputs):
    """Full-input entry: shards over 8 NeuronCores (2 batches each),
    runs the Bass kernel, gathers the full [16, 512, 32, 32] output."""
    from concourse import bass_utils

    if "nc" not in _cache:
        _cache["nc"] = build_program(num_devices=N_CORES)
    nc = _cache["nc"]
    in_maps = prep_inputs(**{k: np.asarray(v) for k, v in inputs.items()})
    res = bass_utils.run_bass_kernel_spmd(
        nc, in_maps, core_ids=list(range(N_CORES)))
    return gather_output(res.results)

